# revision 1
# baseline (speedup 1.0000x reference)
"""CRF loss kernel for Trainium2 (8 NeuronCores, data-parallel over batch).

Math: the log-domain forward recurrence
    alpha_t[i] = logsumexp_j(alpha_{t-1}[j] + trans[i,j]) + feat_t[i]
is run in probability domain:
    P_t = exp(feat_t - c) * (E @ P_{t-1}),   E = exp(trans)
so each step is one tiny 64xW TensorE matmul plus one VectorE multiply.

The serial MM->mul chain is latency-bound (~450ns/link solo), so the
per-core batch of 64 is split into 2 independent chains of 32 columns that
interleave on the engines; the steady state then runs at the DVE's
throughput floor (~320ns/step), since every chain link must cross
PSUM->SBUF on the DVE (GPSIMD cannot read PSUM on real HW) and each DVE
op carries a fixed ~250ns PSUM-access cost. All matmul operands are bf16:
the error budget is huge (tolerance 2e-2 on a loss of magnitude ~1e3),
bf16 shares fp32's exponent range, and a bf16 matmul stays under the PE's
fixed 173ns output latency even at cold clock, where an fp32 one does not.

A constant shift c plus a per-chain renorm every R steps keeps P in range;
renorms are staggered across chains, measured off the critical path
(TensorE ones-matmul + DVE reciprocal), broadcast via a second matmul,
copied to SBUF by the Act engine, and folded into the exp(feat) tile of a
later step by the Pool engine - nothing but the per-step multiply runs on
the DVE. The STOP row of the state history is archived in SBUF, streamed
to DRAM in blocks behind the chain, and the host epilogue picks slot
seq_len[b]+1 per batch. The initial state p0 rides in with the first
feature chunk so startup is two DMAs deep.
"""
import numpy as np

_B, _S, _T = 512, 512, 64
_NCORE = 8
_BC = _B // _NCORE          # 64 batches per core
_G = 2                      # independent interleaved chains per core
_W = _BC // _G              # 32 batch columns per chain
_START, _STOP = 62, 63
_R = 128                    # renorm period (per chain)
_LAG = 4                    # renorm measured at t applies at t+_LAG
_NSTEP = _S + 1             # matmul steps 1..513
_NHIST = _NSTEP + 1         # history slots 0..513 (slot 0 unused)
# chain g renorms at t = _R*(k+1) + 8*g: staggered so aux tiles don't coexist
_RENORM_TG = [[_R * (k + 1) + 8 * g for k in range(3)] for g in range(_G)]
_NEVT = 3
_CH = 8                     # steps per feat DMA/exp chunk
_HBLK = 4 * _CH             # hist63 streaming block (steps)

_cache = {}


def _build_nc():
    import concourse.bass as bass
    import concourse.bacc as bacc
    import concourse.tile as tile
    from concourse import mybir
    from contextlib import ExitStack

    f32 = mybir.dt.float32
    bf16 = mybir.dt.bfloat16
    nc = bacc.Bacc("TRN2", target_bir_lowering=False, debug=False,
                   num_devices=_NCORE)
    # featT block m (m>=1) is step m's features; block 0 is unused padding.
    featT = nc.dram_tensor("featT", [_T, (_NSTEP + 1) * _BC], bf16,
                           kind="ExternalInput").ap()
    # boot = [E.T | p0 | feat_1 | feat_2]: everything steps 1-2 need, one DMA
    boot = nc.dram_tensor("boot", [_T, _T + 3 * _BC], bf16,
                          kind="ExternalInput").ap()
    hist63o = [nc.dram_tensor(f"hist63_{g}", [1, _NHIST * _W], bf16,
                              kind="ExternalOutput").ap() for g in range(_G)]
    sinvo = [nc.dram_tensor(f"sinv_{g}", [1, _NEVT * _W], f32,
                            kind="ExternalOutput").ap() for g in range(_G)]

    with tile.TileContext(nc) as tc, ExitStack() as ctx:
        consts = ctx.enter_context(tc.tile_pool(name="consts", bufs=1))
        fpool = ctx.enter_context(tc.tile_pool(name="fpool", bufs=3))
        epool = ctx.enter_context(tc.tile_pool(name="epool", bufs=3))
        ps_g = [ctx.enter_context(
            tc.tile_pool(name=f"ps{g}", bufs=2, space="PSUM"))
            for g in range(_G)]
        ps_aux = ctx.enter_context(tc.tile_pool(name="ps_aux", bufs=2,
                                                space="PSUM"))
        bcpool = ctx.enter_context(tc.tile_pool(name="bcpool", bufs=2))

        # one boot DMA delivers E, p0, and the first two steps' features
        boot_sb = consts.tile([_T, _T + 3 * _BC], bf16)
        nc.sync.dma_start(boot_sb[:, :], boot)
        E_sb = boot_sb[:, 0:_T]
        ones_k = consts.tile([_T, 1], bf16)
        nc.vector.memset(ones_k[:, :], 1.0)
        ones_m = consts.tile([1, _T], f32)
        nc.vector.memset(ones_m[:, :], 1.0)

        hists = [consts.tile([_T, _NHIST * _W], bf16, name=f"hist{g}")
                 for g in range(_G)]
        sinvs = [consts.tile([1, _NEVT * _W], f32, name=f"sinv{g}")
                 for g in range(_G)]


        renorm_at = {}
        fold_at = {}
        for g in range(_G):
            for e, te in enumerate(_RENORM_TG[g]):
                renorm_at[(te, g)] = e
                fold_at[(te + _LAG, g)] = e
        last_evt_t = max(te for g in range(_G) for te in _RENORM_TG[g])

        curs = [boot_sb[:, _T + g * _W:_T + (g + 1) * _W] for g in range(_G)]
        evt_bc = {}
        sinv_sent = False
        hist_sent = [_W] * _G                # hist63 cols streamed (slot 0 unused)
        t = 1
        while t <= _NSTEP:
            if t == 1:
                n_t = 2
                fch_f = boot_sb[:, _T + _BC:]
            else:
                n_t = min(_CH, _NSTEP - t + 1)
                fchunk = fpool.tile([_T, _CH * _BC], bf16, tag="fchunk")
                nc.sync.dma_start(
                    fchunk[:, : n_t * _BC],
                    featT[:, t * _BC: (t + n_t) * _BC],
                )
                fch_f = fchunk[:, : n_t * _BC]
            Fch = epool.tile([_T, _CH * _BC], bf16, tag="Fch")
            nc.scalar.activation(
                Fch[:, : n_t * _BC], fch_f,
                mybir.ActivationFunctionType.Exp,
            )
            for k in range(n_t):
                for g in range(_G):
                    fsl = Fch[:, k * _BC + g * _W: k * _BC + (g + 1) * _W]
                    if (t, g) in fold_at:
                        # apply the pending renorm scale to this step's F
                        bc = evt_bc.pop((fold_at[(t, g)], g))
                        nc.gpsimd.tensor_mul(fsl, fsl, bc[:, :])
                    ps = ps_g[g].tile([_T, _W], f32, tag=f"ps{g}")
                    nc.tensor.matmul(ps[:, :], E_sb[:, :], curs[g],
                                     start=True, stop=True)
                    dst = hists[g][:, t * _W: (t + 1) * _W]
                    nc.vector.tensor_mul(dst, ps[:, :], fsl)
                    curs[g] = dst
                    if (t, g) in renorm_at:
                        e = renorm_at[(t, g)]
                        s_ps = ps_aux.tile([1, _W], f32, tag="s_ps")
                        nc.tensor.matmul(s_ps[:, :], ones_k[:, :], dst,
                                         start=True, stop=True)
                        s_sb = bcpool.tile([1, _W], f32, tag="s_sb")
                        nc.scalar.copy(s_sb[:, :], s_ps[:, :])
                        sv = sinvs[g][:, e * _W: (e + 1) * _W]
                        nc.vector.reciprocal(sv, s_sb[:, :])
                        bc_ps = ps_aux.tile([_T, _W], f32, tag="bc_ps")
                        nc.tensor.matmul(bc_ps[:, :], ones_m[:, :], sv,
                                         start=True, stop=True)
                        bc = bcpool.tile([_T, _W], f32, tag="bc")
                        nc.scalar.copy(bc[:, :], bc_ps[:, :])
                        evt_bc[(e, g)] = bc
                t += 1
            # stream completed hist63 blocks out behind the chain; flush every
            # chunk near the end so the final post-chain DMA is tiny
            if t - hist_sent[0] // _W > _HBLK or t > _NSTEP - 2 * _HBLK:
                for g in range(_G):
                    lo, hi = hist_sent[g], t * _W
                    # final flushes go out on two different DGE queues
                    eng = nc.scalar if (t > _NSTEP and g == 0) else nc.sync
                    eng.dma_start(hist63o[g][:, lo:hi],
                                  hists[g][_STOP:_STOP + 1, lo:hi])
                    hist_sent[g] = hi
            if t > last_evt_t and not sinv_sent:
                sinv_sent = True
                for g in range(_G):
                    nc.sync.dma_start(sinvo[g], sinvs[g][:, :])
    nc.compile()
    return nc


def _prep_inputs(feas, transitions):
    import ml_dtypes

    E = np.exp(transitions.astype(np.float32))
    rows = np.ones(_T, bool)
    rows[_START] = False
    c = float(np.log(E.sum(1)[rows]).mean())
    lhsT = np.ascontiguousarray(E.T).astype(ml_dtypes.bfloat16)  # lhsT[j,i]=E[i,j]

    # featT per core: [T, (1+NSTEP)*BC]; block 0 = p0, block m>=1 is step m's
    # feat: featT[i, m*BC + b] = feas[b0+b, m-1, i] - c for m<=S; step 513 -> -c.
    ft = np.transpose(feas.astype(np.float32), (2, 1, 0)) - np.float32(c)  # [T,S,B]
    in_maps = []
    for cix in range(_NCORE):
        sl = ft[:, :, cix * _BC: (cix + 1) * _BC]                       # [T,S,BC]
        full = np.empty((_T, _NSTEP + 1, _BC), np.float32)
        p0 = np.zeros((_T, _BC), np.float32)
        p0[_START, :] = 1.0
        full[:, 0, :] = p0
        full[:, 1:_S + 1, :] = sl
        full[:, _S + 1, :] = -c
        featT_bf = np.ascontiguousarray(
            full.reshape(_T, (_NSTEP + 1) * _BC)).astype(ml_dtypes.bfloat16)
        in_maps.append({
            "featT": featT_bf,
            "boot": np.ascontiguousarray(
                np.hstack([lhsT, featT_bf[:, 0:3 * _BC]])),
        })
    return c, in_maps


def kernel(feas, transitions, tag, seq_len):
    from concourse.bass_utils import run_bass_kernel_spmd

    feas = np.asarray(feas)
    transitions = np.asarray(transitions)
    tag = np.asarray(tag)
    seq_len = np.asarray(seq_len)

    if "nc" not in _cache:
        _cache["nc"] = _build_nc()
    nc = _cache["nc"]

    c, in_maps = _prep_inputs(feas, transitions)
    res = run_bass_kernel_spmd(nc, in_maps, list(range(_NCORE))).results

    # ---- host epilogue: norm from archived history ----
    L = seq_len.astype(np.int64)                                        # [B]
    # batch column b = cix*BC + g*W + w
    hist63 = np.concatenate(
        [res[cix][f"hist63_{g}"].reshape(_NHIST, _W).astype(np.float64)
         for cix in range(_NCORE) for g in range(_G)], axis=1
    )                                                                   # [NHIST, B]
    sinv = np.concatenate(
        [res[cix][f"sinv_{g}"].reshape(_NEVT, _W).astype(np.float64)
         for cix in range(_NCORE) for g in range(_G)], axis=1
    )                                                                   # [NEVT, B]
    tevt = np.concatenate(
        [np.asarray(_RENORM_TG[g])[:, None].repeat(_W, 1)
         for _ in range(_NCORE) for g in range(_G)], axis=1
    )                                                                   # [NEVT, B]
    # scale 1/s_e is folded into F of step t_e+_LAG, so it is present in
    # hist slot m for m >= t_e+_LAG; capture slot is m = L+1.
    logsum = np.where(tevt + _LAG <= (L + 1)[None, :],
                      -np.log(sinv), 0.0).sum(0)
    featT_val = np.where(
        L < _S,
        feas[np.arange(_B), np.minimum(L, _S - 1), _STOP].astype(np.float64) - c,
        -c,
    )
    norm = c * L + logsum + np.log(hist63[L + 1, np.arange(_B)]) - featT_val

    # ---- gold score ----
    dt = np.float32
    pos = np.arange(_S + 2)
    lbl = np.concatenate(
        [np.full((_B, 1), _START, tag.dtype), tag, np.full((_B, 1), _STOP, tag.dtype)],
        axis=1,
    )
    lbl = np.where(pos[None, :] <= L[:, None], lbl, _STOP)
    trn = transitions[lbl[:, 1:], lbl[:, :-1]]
    tmask = (np.arange(_S + 1)[None, :] <= L[:, None]).astype(dt)
    trans_score = (trn.astype(dt) * tmask).sum(1)
    emit = np.take_along_axis(feas, tag[..., None], axis=2)[..., 0]
    emask = (np.arange(_S)[None, :] < L[:, None]).astype(dt)
    emit_score = (emit.astype(dt) * emask).sum(1)

    return (norm - (trans_score + emit_score)).astype(np.float32)



# revision 4
# speedup vs baseline: 1.1054x; 1.1054x over previous
"""CRF loss kernel for Trainium2 (8 NeuronCores, data-parallel over batch).

Math: the log-domain forward recurrence
    alpha_t[i] = logsumexp_j(alpha_{t-1}[j] + trans[i,j]) + feat_t[i]
is run in probability domain:
    P_t = exp(feat_t - c) * (E @ P_{t-1}),   E = exp(trans)
so each step is one tiny matmul plus one VectorE multiply.

Layout: T=64 tags use only half the 128 SBUF partitions, and the cost of a
DVE/matmul op depends only on its FREE size, so two 32-column batch groups
are STACKED on the partition axis (partitions 0:64 = cols 0:16, 64:128 =
cols 16:32 of the group) and advanced by a block-diagonal 128x128 transition
matrix. One matmul + one DVE multiply then serve 32 batch columns at free
width 16. Two such superchains interleave; steady state is DVE-bound at
~284ns/step = 2 x (125ns PSUM-access bubble + 16x1.04ns), just under the
serial link latency (~100ns matmul visibility + 45ns DVE decode + DVE busy).

All matmul operands are bf16 (error budget is huge: tolerance 2e-2 on a
loss of magnitude ~1e3; bf16 shares fp32's exponent range). A constant
shift c plus a per-superchain renorm every 128 steps keeps P in range;
renorms are staggered across superchains, measured off the critical path
(TensorE 2-row ones-matmul + DVE reciprocal), broadcast back to 128 rows
via a second matmul (2-partition stationary), copied to SBUF by the Act
engine, and folded into the exp(feat) tile of a later step by the Pool
engine. The STOP rows (partitions 63 and 127) of the state history are
archived in SBUF, streamed to DRAM in blocks behind the chain, and the
host epilogue picks slot seq_len[b]+1 per batch column.
"""
import numpy as np

_B, _S, _T = 512, 512, 64
_NCORE = 8
_BC = _B // _NCORE          # 64 batches per core
_G = 2                      # superchains per core
_WG = _BC // _G             # 32 batch columns per superchain
_R2 = 2                     # column groups stacked per superchain
_W = _WG // _R2             # 16 free columns per superchain tile
_P = 128                    # partitions
_START, _STOP = 62, 63
_R = 128                    # renorm period (per superchain)
_LAG = 4                    # renorm measured at t applies at t+_LAG
_NSTEP = _S + 1             # matmul steps 1..513
_NHIST = _NSTEP + 1         # history slots 0..513 (slot 0 unused)
# superchain g renorms at t = _R*(k+1) + 8*g: staggered so aux tiles don't
# coexist
_RENORM_TG = [[_R * (k + 1) + 8 * g for k in range(3)] for g in range(_G)]
_NEVT = 3
_CH = 8                     # steps per feat DMA/exp chunk
_HBLK = 4 * _CH             # hist63 streaming block (steps)

# boot column layout: [E2T | ones2 | Bm | featT2 steps 0..2]
_BOOT_E = 0                  # E2T at cols 0:128
_BOOT_ONES = _P              # ones2 at cols 128:130
_BOOT_B = _P + 2             # Bm (rows 0:2) at cols 130:258
_BOOT_F = _P + 2 + _P        # featT2[:, 0:96] at cols 258:354
_BOOT_COLS = _BOOT_F + 3 * _G * _W

_cache = {}


def _build_nc():
    import concourse.bass as bass
    import concourse.bacc as bacc
    import concourse.tile as tile
    from concourse import mybir
    from contextlib import ExitStack

    f32 = mybir.dt.float32
    bf16 = mybir.dt.bfloat16
    GW = _G * _W            # 32 = feature columns per step
    nc = bacc.Bacc("TRN2", target_bir_lowering=False, debug=False,
                   num_devices=_NCORE)
    # featT2 block m (m>=1) is step m's features; block 0 is p0.
    featT2 = nc.dram_tensor("featT2", [_P, (_NSTEP + 1) * GW], bf16,
                            kind="ExternalInput").ap()
    boot = nc.dram_tensor("boot", [_P, _BOOT_COLS], bf16,
                          kind="ExternalInput").ap()
    bootf = nc.dram_tensor("bootf", [_R2, _P], f32,
                           kind="ExternalInput").ap()
    hist63o = [nc.dram_tensor(f"hist63_{g}", [_R2, _NHIST * _W], bf16,
                              kind="ExternalOutput").ap() for g in range(_G)]
    sinvo = [nc.dram_tensor(f"sinv_{g}", [_R2, _NEVT * _W], f32,
                            kind="ExternalOutput").ap() for g in range(_G)]

    with tile.TileContext(nc) as tc, ExitStack() as ctx:
        consts = ctx.enter_context(tc.tile_pool(name="consts", bufs=1))
        fpool = ctx.enter_context(tc.tile_pool(name="fpool", bufs=3))
        epool = ctx.enter_context(tc.tile_pool(name="epool", bufs=3))
        ps_g = [ctx.enter_context(
            tc.tile_pool(name=f"ps{g}", bufs=2, space="PSUM"))
            for g in range(_G)]
        ps_aux = ctx.enter_context(tc.tile_pool(name="ps_aux", bufs=2,
                                                space="PSUM"))
        bcpool = ctx.enter_context(tc.tile_pool(name="bcpool", bufs=2))

        # one boot DMA delivers E2T, ones2, Bm, p0, and steps 1-2's features
        boot_sb = consts.tile([_P, _BOOT_COLS], bf16, name="boot_sb")
        nc.sync.dma_start(boot_sb[:, :], boot)
        E2_sb = boot_sb[:, _BOOT_E:_BOOT_E + _P]
        ones2 = boot_sb[:, _BOOT_ONES:_BOOT_ONES + _R2]
        Bm_sb = consts.tile([_R2, _P], f32, name="Bm_sb")
        nc.sync.dma_start(Bm_sb[:, :], bootf)
        Bm = Bm_sb[:, :]

        hists = [consts.tile([_P, _NHIST * _W], bf16, name=f"hist{g}")
                 for g in range(_G)]
        sinvs = [consts.tile([_R2, _NEVT * _W], f32, name=f"sinv{g}")
                 for g in range(_G)]

        renorm_at = {}
        fold_at = {}
        for g in range(_G):
            for e, te in enumerate(_RENORM_TG[g]):
                renorm_at[(te, g)] = e
                fold_at[(te + _LAG, g)] = e
        last_evt_t = max(te for g in range(_G) for te in _RENORM_TG[g])

        curs = [boot_sb[:, _BOOT_F + g * _W:_BOOT_F + (g + 1) * _W]
                for g in range(_G)]
        evt_bc = {}
        sinv_sent = False
        hist_sent = [_W] * _G            # hist63 cols streamed (slot 0 unused)
        t = 1
        while t <= _NSTEP:
            if t == 1:
                n_t = 2
                fch_f = boot_sb[:, _BOOT_F + GW:]
            else:
                n_t = min(_CH, _NSTEP - t + 1)
                fchunk = fpool.tile([_P, _CH * GW], bf16, tag="fchunk")
                nc.sync.dma_start(
                    fchunk[:, : n_t * GW],
                    featT2[:, t * GW: (t + n_t) * GW],
                )
                fch_f = fchunk[:, : n_t * GW]
            Fch = epool.tile([_P, _CH * GW], bf16, tag="Fch")
            nc.scalar.activation(
                Fch[:, : n_t * GW], fch_f,
                mybir.ActivationFunctionType.Exp,
            )
            for k in range(n_t):
                for g in range(_G):
                    fsl = Fch[:, k * GW + g * _W: k * GW + (g + 1) * _W]
                    if (t, g) in fold_at:
                        # apply the pending renorm scale to this step's F
                        bc = evt_bc.pop((fold_at[(t, g)], g))
                        nc.gpsimd.tensor_mul(fsl, fsl, bc[:, :])
                    ps = ps_g[g].tile([_P, _W], f32, tag=f"ps{g}")
                    nc.tensor.matmul(ps[:, :], E2_sb, curs[g],
                                     start=True, stop=True)
                    dst = hists[g][:, t * _W: (t + 1) * _W]
                    nc.vector.tensor_mul(dst, ps[:, :], fsl)
                    curs[g] = dst
                    if (t, g) in renorm_at:
                        e = renorm_at[(t, g)]
                        s_ps = ps_aux.tile([_R2, _W], f32, tag="s_ps")
                        nc.tensor.matmul(s_ps[:, :], ones2, dst,
                                         start=True, stop=True)
                        s_sb = bcpool.tile([_R2, _W], f32, tag="s_sb")
                        nc.scalar.copy(s_sb[:, :], s_ps[:, :])
                        sv = sinvs[g][:, e * _W: (e + 1) * _W]
                        nc.vector.reciprocal(sv, s_sb[:, :])
                        bc_ps = ps_aux.tile([_P, _W], f32, tag="bc_ps")
                        nc.tensor.matmul(bc_ps[:, :], Bm, sv,
                                         start=True, stop=True)
                        bc = bcpool.tile([_P, _W], f32, tag="bc")
                        nc.scalar.copy(bc[:, :], bc_ps[:, :])
                        evt_bc[(e, g)] = bc
                t += 1
            # stream completed hist63 blocks out behind the chain; flush every
            # chunk near the end so the final post-chain DMA is tiny
            if t - hist_sent[0] // _W > _HBLK or t > _NSTEP - 2 * _HBLK:
                for g in range(_G):
                    lo, hi = hist_sent[g], t * _W
                    # final flushes go out on two different DGE queues
                    eng = nc.scalar if (t > _NSTEP and g == 0) else nc.sync
                    eng.dma_start(hist63o[g][0:1, lo:hi],
                                  hists[g][_STOP:_STOP + 1, lo:hi])
                    eng.dma_start(hist63o[g][1:2, lo:hi],
                                  hists[g][_T + _STOP:_T + _STOP + 1, lo:hi])
                    hist_sent[g] = hi
            if t > last_evt_t and not sinv_sent:
                sinv_sent = True
                for g in range(_G):
                    nc.sync.dma_start(sinvo[g], sinvs[g][:, :])
    nc.compile()
    return nc


def _prep_inputs(feas, transitions):
    import ml_dtypes

    E = np.exp(transitions.astype(np.float32))
    rows = np.ones(_T, bool)
    rows[_START] = False
    c = float(np.log(E.sum(1)[rows]).mean())
    ET = np.ascontiguousarray(E.T).astype(np.float32)  # ET[j,i]=E[i,j]
    E2T = np.zeros((_P, _P), np.float32)
    E2T[:_T, :_T] = ET
    E2T[_T:, _T:] = ET
    ones2 = np.zeros((_P, _R2), np.float32)
    ones2[:_T, 0] = 1.0
    ones2[_T:, 1] = 1.0
    Bpad = np.zeros((_P, _P), np.float32)       # rows 0:2 hold Bm
    Bpad[0, :_T] = 1.0
    Bpad[1, _T:] = 1.0

    # featT2 per core: [P, (1+NSTEP)*GW]; block 0 = p0 (stacked one-hot at
    # START), block m>=1 is step m's features shifted by -c; step 513 -> -c.
    ft = np.transpose(feas.astype(np.float32), (2, 1, 0)) - np.float32(c)  # [T,S,B]
    GW = _G * _W
    in_maps = []
    for cix in range(_NCORE):
        sl = ft[:, :, cix * _BC: (cix + 1) * _BC]                  # [T,S,BC]
        full = np.empty((_T, _NSTEP + 1, _BC), np.float32)
        p0 = np.zeros((_T, _BC), np.float32)
        p0[_START, :] = 1.0
        full[:, 0, :] = p0
        full[:, 1:_S + 1, :] = sl
        full[:, _S + 1, :] = -c
        # stack: featT2[p, m*GW + g*W + n] = full[p%T, m, g*WG + (p//T)*W + n]
        fu = full.reshape(_T, _NSTEP + 1, _G, _R2, _W)             # [T,m,g,r,n]
        featT2 = np.concatenate([fu[:, :, :, 0, :], fu[:, :, :, 1, :]],
                                axis=0)                            # [P,m,g,n]
        featT2 = np.ascontiguousarray(
            featT2.reshape(_P, (_NSTEP + 1) * GW)).astype(ml_dtypes.bfloat16)
        boot = np.hstack([
            E2T, ones2, Bpad, featT2[:, 0:3 * GW].astype(np.float32),
        ]).astype(ml_dtypes.bfloat16)
        in_maps.append({
            "featT2": featT2,
            "boot": np.ascontiguousarray(boot),
            "bootf": np.ascontiguousarray(Bpad[0:_R2, :]),
        })
    return c, in_maps


def kernel(feas, transitions, tag, seq_len):
    from concourse.bass_utils import run_bass_kernel_spmd

    feas = np.asarray(feas)
    transitions = np.asarray(transitions)
    tag = np.asarray(tag)
    seq_len = np.asarray(seq_len)

    if "nc" not in _cache:
        _cache["nc"] = _build_nc()
    nc = _cache["nc"]

    c, in_maps = _prep_inputs(feas, transitions)
    res = run_bass_kernel_spmd(nc, in_maps, list(range(_NCORE))).results

    # ---- host epilogue: norm from archived history ----
    L = seq_len.astype(np.int64)                                        # [B]
    # batch column b = cix*BC + g*WG + r*W + n
    hist63 = np.concatenate(
        [res[cix][f"hist63_{g}"].reshape(_R2, _NHIST, _W)
         .transpose(1, 0, 2).reshape(_NHIST, _WG).astype(np.float64)
         for cix in range(_NCORE) for g in range(_G)], axis=1
    )                                                                   # [NHIST, B]
    sinv = np.concatenate(
        [res[cix][f"sinv_{g}"].reshape(_R2, _NEVT, _W)
         .transpose(1, 0, 2).reshape(_NEVT, _WG).astype(np.float64)
         for cix in range(_NCORE) for g in range(_G)], axis=1
    )                                                                   # [NEVT, B]
    tevt = np.concatenate(
        [np.asarray(_RENORM_TG[g])[:, None].repeat(_WG, 1)
         for _ in range(_NCORE) for g in range(_G)], axis=1
    )                                                                   # [NEVT, B]
    # scale 1/s_e is folded into F of step t_e+_LAG, so it is present in
    # hist slot m for m >= t_e+_LAG; capture slot is m = L+1.
    logsum = np.where(tevt + _LAG <= (L + 1)[None, :],
                      -np.log(sinv), 0.0).sum(0)
    featT_val = np.where(
        L < _S,
        feas[np.arange(_B), np.minimum(L, _S - 1), _STOP].astype(np.float64) - c,
        -c,
    )
    norm = c * L + logsum + np.log(hist63[L + 1, np.arange(_B)]) - featT_val

    # ---- gold score ----
    dt = np.float32
    pos = np.arange(_S + 2)
    lbl = np.concatenate(
        [np.full((_B, 1), _START, tag.dtype), tag, np.full((_B, 1), _STOP, tag.dtype)],
        axis=1,
    )
    lbl = np.where(pos[None, :] <= L[:, None], lbl, _STOP)
    trn = transitions[lbl[:, 1:], lbl[:, :-1]]
    tmask = (np.arange(_S + 1)[None, :] <= L[:, None]).astype(dt)
    trans_score = (trn.astype(dt) * tmask).sum(1)
    emit = np.take_along_axis(feas, tag[..., None], axis=2)[..., 0]
    emask = (np.arange(_S)[None, :] < L[:, None]).astype(dt)
    emit_score = (emit.astype(dt) * emask).sum(1)

    return (norm - (trans_score + emit_score)).astype(np.float32)
